# revision 10
# baseline (speedup 1.0000x reference)
"""Trainium2 Bass kernel for the Antenna message-generation MLP.

Reference computation (per batch b, RF-chain r, antenna u):
    x[b,r,u,:48] = concat(F[b,:,r], sum_u C[b,u,r,:], H[b,u,8r:8r+8], H[b,u,64+8r:64+8r+8])
    out[b,r,u,:] = tanh(relu(relu(x@W1+b1)@W2+b2)@W3+b3)

Strategy: pure data parallelism over batch across 8 NeuronCores (256
batches = 16384 rows per core).  Rows are processed in 1024-row chunks
(two 512-row subtiles A/B), activations feature-on-partition, fp16 on
the PE (fp32 psum).

Differences from the previous 198us version:
  * Weights are packed on the HOST into fp16 device layouts (w1p/w2p/
    w3p + one [128,9] bias pack) -- no SWDGE cast DMAs, no on-chip w1
    shuffling, and b1/b2/b3 ride the ACT bias port so the folded-bias
    ones rows disappear (L1 contraction 48 in a 64-row band).
  * L1 is 2-way ROW-TILED: subtile A's X^T at partitions 0:64 with the
    stationary at array rows 0:64, subtile B at 64:128/(64,0).  The two
    64-contraction matmuls run concurrently on disjoint row bands ->
    half the PE slots of the old zero-padded 128x128 scheme.
  * Emission interleaves each L1 pair with two L2 groups of the
    previous chunk so psum-bank WAR never blocks the PE FIFO head.
  * PSUM: L1 2x two-bank pair tiles, L2 3 banks (the old 2-bank L2
    rotation cost +54ns at every 4-MM group boundary), L3 packs its 4
    column bands (partitions 32j) into ONE bank.
  * Evacuations balanced across scalar/ACT and vector/DVE (Pool can't
    read PSUM): scalar 7 L2 evacs + 2 pair evacs + a tanh half per
    chunk, DVE 2 pair evacs + 1 L2 evac + builds/transposes.
  * Ramp: chunk 0-3 input DMAs spread across sync/vector/scalar/gpsimd
    queues; tail: final group's tanh/store of bands 0:64 overlaps the
    last chunk's L2 groups.

X^T strip layout (per 64-partition half):
    [0:16)=h  [16:32)=zeros  [32:48)=c  [48:64)=F
C/H land via one merged [128,512] DMA + one DVE 32x32 stream transpose;
c is u-summed by a single tensor_reduce and rejoined with DMA-transposed
F in a 32-row fc tile so one broadcast copy fills c+F per strip.
"""

import sys
import types

import numpy as np

# This image's `antenv` lacks `axon_hooks`; bass_utils imports it when
# BASS_TRACE is set.  Register a no-op stand-in so tracing degrades
# gracefully instead of crashing (real hook installed by test harness).
try:
    import antenv.axon_hooks  # noqa: F401
except ImportError:
    import antenv

    _m = types.ModuleType("antenv.axon_hooks")
    _m._hook = None
    _m.set_axon_ntff_profile_hook = lambda h: setattr(_m, "_hook", h)
    _m.get_axon_ntff_profile_hook = lambda: _m._hook
    sys.modules["antenv.axon_hooks"] = _m
    antenv.axon_hooks = _m

import concourse.bacc as bacc
import concourse.mybir as mybir
import concourse.tile as tile
from concourse.bass_utils import run_bass_kernel_spmd

F32 = mybir.dt.float32
F16 = mybir.dt.float16

N_CORES = 8
B_FULL = 2048
B_SH = B_FULL // N_CORES    # 256 batches per core
U = 8
R = 8
M = 16
FDIM = 16
H1 = 512
H2 = 512

BG = 16                     # batches per build chunk (1024 rows)
NCH = B_SH // BG            # 16 chunks per core
TILE = 512                  # rows per subtile / psum bank of fp32

N_WARM = 128                # PE warm-up matmuls before first L1 pair
N_FILL = 96                 # pipeline-fill matmuls after chunk 0's pairs

_CACHE = {}


def _build():
    nc = bacc.Bacc("TRN2", target_bir_lowering=False, debug=False)

    C_ext = nc.dram_tensor("C", [B_SH, U, R, M], F32, kind="ExternalInput")
    F_ext = nc.dram_tensor("F", [B_SH, FDIM, R], F32, kind="ExternalInput")
    H_ext = nc.dram_tensor("H", [B_SH, U, 2 * 64], F32, kind="ExternalInput")
    # host-packed weights (see _pack_weights)
    w1_ext = nc.dram_tensor("w1p", [128, H1], F16, kind="ExternalInput")
    w2_ext = nc.dram_tensor("w2p", [128, 4, H2], F16, kind="ExternalInput")
    w3_ext = nc.dram_tensor("w3p", [128, 4, 32], F16, kind="ExternalInput")
    # cols 0:4 = b1 (by s-tile), 4:8 = b2 (by t-tile), 8 = b3 (banded)
    bias_ext = nc.dram_tensor("biasp", [128, 9], F32, kind="ExternalInput")
    out_ext = nc.dram_tensor("out", [B_SH, R, U, M], F32, kind="ExternalOutput")

    out_rows = out_ext.ap().rearrange("b r u m -> (b r u) m")  # [16384, 16]

    relu = mybir.ActivationFunctionType.Relu
    tanh = mybir.ActivationFunctionType.Tanh
    axis_x = mybir.AxisListType.X
    op_add = mybir.AluOpType.add
    op_max = mybir.AluOpType.max

    with tile.TileContext(nc) as tc:
        with (
            tc.tile_pool(name="consts", bufs=1) as consts,
            tc.tile_pool(name="loads", bufs=6) as loads,
            tc.tile_pool(name="mts", bufs=3) as mts,
            tc.tile_pool(name="fcs", bufs=6) as fcs,
            tc.tile_pool(name="a1s", bufs=3) as a1p,
            tc.tile_pool(name="a2s", bufs=4) as a2p,
            tc.tile_pool(name="outs", bufs=2) as outs,
            tc.tile_pool(name="p1", bufs=2, space="PSUM") as p1p,
            tc.tile_pool(name="p2", bufs=3, space="PSUM") as p2p,
            tc.tile_pool(name="py", bufs=1, space="PSUM") as pyp,
        ):
            # ---- persistent tiles --------------------------------------
            w1 = consts.tile([128, H1], F16)
            w2 = consts.tile([128, 4, H2], F16)
            w3 = consts.tile([128, 4, 32], F16)
            biasc = consts.tile([128, 9], F32)
            wtile = consts.tile([128, 128], F16)
            wscr = consts.tile([128, 1], F32)
            xts = [consts.tile([128, TILE], F16, tag=f"xt{i}", name=f"xt{i}")
                   for i in range(4)]

            mpads = []
            fc_tiles = []

            def build_dma(c, qc=None, qh=None, qf=None):
                qc = qc or nc.sync
                qh = qh or nc.sync
                qf = qf or nc.sync
                b0 = c * BG
                mp = loads.tile([128, 512], F32, tag="mpad", name="mpad")
                # the DMAs below only fill the lower 16 cols of each 32-col
                # r-block; zero the upper halves so the full-tile DVE
                # transpose never reads uninitialized SBUF (the transposed
                # garbage bands are discarded, but CoreSim checks reads)
                mpv = mp.rearrange("p (r w m) -> p r w m", r=2 * R, w=2)
                nc.gpsimd.memset(mpv[:, :, 1, :], 0.0)
                # c-region: cols 32r + m (m<16)
                qc.dma_start(
                    mp[:, 0:256].rearrange("p (r w) -> p r w", r=R)[:, :, 0:M],
                    C_ext[b0 : b0 + BG].rearrange("b u r m -> (b u) r m"),
                )
                # h-region: cols 256 + 32r + 8i + k
                hp_v = mp[:, 256:512].rearrange("p (r w) -> p r w", r=R)
                h_src = H_ext[b0 : b0 + BG].rearrange(
                    "b u (i r k) -> (b u) i r k", i=2, r=R
                )
                for i in range(2):
                    qh.dma_start(hp_v[:, :, 8 * i : 8 * i + 8], h_src[:, i])
                # F slice straight into fc rows 16:32 (DMA writes any base)
                fcv = fcs.tile([32, 128], F32, tag="fc", name="fc")
                qf.dma_start(
                    fcv[16:32, :].rearrange("f (b r) -> f b r", b=BG),
                    F_ext[b0 : b0 + BG].rearrange("b f r -> f b r"),
                )
                mpads.append(mp)
                fc_tiles.append(fcv)

            # ---- ramp: all input + weight DMAs first, spread over the
            # three DMA-capable queues (sync/SP, scalar/ACT, gpsimd) ----
            # xt pad rows must be finite zeros once; memsets first on the
            # gpsimd queue so build(0)'s xt writes aren't stuck behind its
            # SWDGE descriptor generation
            nc.gpsimd.memset(wtile[:], 0.0)
            for xt in xts:
                nc.gpsimd.memset(xt[:], 0.0)
            build_dma(0, qc=nc.sync, qh=nc.scalar, qf=nc.scalar)
            nc.sync.dma_start(w1[:], w1_ext.ap())
            build_dma(1, qc=nc.scalar, qh=nc.sync, qf=nc.sync)
            # hoist the ~2.7us ACT table load off the critical path
            nc.scalar.activation(wscr[:], wtile[:, 0:1], tanh)
            nc.scalar.dma_start(w2[:], w2_ext.ap())
            nc.sync.dma_start(w3[:], w3_ext.ap())
            nc.sync.dma_start(biasc[:], bias_ext.ap())
            build_dma(2, qc=nc.sync, qh=nc.sync, qf=nc.scalar)
            build_dma(3, qc=nc.sync, qh=nc.scalar, qf=nc.scalar)

            # ---- PE warm-up: keep HAM busy through the input ramp ------
            ps_warm = pyp.tile([128, TILE], F32, tag="psy", name="ps_warm")

            def warm(n):
                for _ in range(n):
                    nc.tensor.matmul(
                        ps_warm[:, 0:64], wtile[:], wtile[:, 0:64],
                        start=True, stop=True,
                    )

            warm(N_WARM)

            # ---- per-chunk build ---------------------------------------
            a1_of_chunk = [None] * NCH
            a2_of_chunk = [None] * NCH
            psy_of_group = [None] * (NCH // 2)
            yt_of_group = [None] * (NCH // 2)

            def build_xt(c):
                mp = mpads[c]
                fcv = fc_tiles[c]
                xt = xts[c % 4]
                mt = mts.tile([128, 512], F32, tag="mt", name="mt")
                nc.vector.transpose(mt[:], mp[:])
                # u-sum of c across all four 32-row bands at once
                cr = mts.tile([128, 32], F32, tag="cred", name="cred")
                nc.vector.tensor_reduce(
                    cr[:],
                    mt[:, 0:256].rearrange("p (rb u) -> p rb u", u=U),
                    axis_x, op_add,
                )
                # c bands -> fc rows 0:16 (cols (b,r) b-major)
                for a in range(4):
                    nc.vector.tensor_copy(
                        fcv[0:16, 32 * a : 32 * a + 32].rearrange(
                            "p (b4 r) -> p r b4", b4=4
                        ),
                        cr[32 * a : 32 * a + 16, :].rearrange(
                            "p (r b4) -> p r b4", b4=4
                        ),
                    )
                # h bands -> xt[0:16) / xt[64:80) -- on gpsimd: these are
                # SBUF->SBUF (Pool can't read PSUM but copies are fine) and
                # the Pool queue is otherwise idle, which unloads the DVE
                for a in range(4):
                    hb = 0 if a < 2 else 64
                    dst = xt[hb : hb + 16, :].rearrange(
                        "p (b r u) -> p b r u", b=8, r=R
                    )[:, 4 * (a & 1) : 4 * (a & 1) + 4]
                    src = mt[32 * a : 32 * a + 16, 256:512].rearrange(
                        "p (r b4 u) -> p b4 r u", b4=4, u=U
                    )
                    nc.gpsimd.tensor_copy(dst, src)
                # fc ([c;F], 32 rows) broadcast over u -> xt[32:64)/[96:128)
                for half in range(2):
                    nc.gpsimd.tensor_copy(
                        xt[32 + 64 * half : 64 + 64 * half, :].rearrange(
                            "p (b r u) -> p b r u", b=8, r=R
                        ),
                        fcv[:, 64 * half : 64 * half + 64]
                        .rearrange("p (b r) -> p b r", b=8)
                        .unsqueeze(3)
                        .broadcast_to((32, 8, R, U)),
                    )

            # ---- L1: 2-way row-tiled pair ------------------------------
            def pair(c, s):
                xt = xts[c % 4]
                psp = p1p.tile([128, 2, TILE], F32, tag="ps1", name="psp")
                for half in range(2):
                    pb = 64 * half
                    nc.tensor.matmul(
                        psp[:, half, :],
                        w1[pb : pb + 64, s * 128 : (s + 1) * 128],
                        xt[pb : pb + 64, :],
                        start=True, stop=True,
                    )
                return psp

            def evac_pair(c, s, psp, eng="S"):
                a1c = a1_of_chunk[c]
                if eng == "S":
                    nc.scalar.activation(
                        a1c[:, s, :, :], psp[:, :, :], relu,
                        bias=biasc[:, s : s + 1],
                    )
                else:
                    nc.vector.tensor_scalar(
                        a1c[:, s, :, :], psp[:, :, :],
                        biasc[:, s : s + 1], 0.0, op_add, op_max,
                    )

            # ---- L2: group k = (h, t), 4-MM accumulation ---------------
            # (gpsimd/Pool cannot touch PSUM on trn2, so evacuations are
            # spread over scalar/ACT and vector/DVE only)
            def l2_group(c, k, eng="S"):
                h, t = k // 4, k % 4
                a1c = a1_of_chunk[c]
                ps2 = p2p.tile([128, TILE], F32, tag="ps2", name="ps2")
                for s in range(4):
                    nc.tensor.matmul(
                        ps2[:],
                        w2[:, s, t * 128 : (t + 1) * 128],
                        a1c[:, s, h, :],
                        start=(s == 0), stop=(s == 3),
                    )
                a2c = a2_of_chunk[c]
                dst = a2c[:, h, t, :]
                bcol = biasc[:, 4 + t : 5 + t]
                if eng == "S":
                    nc.scalar.activation(dst, ps2[:], relu, bias=bcol)
                else:
                    nc.vector.tensor_scalar(
                        dst, ps2[:], bcol, 0.0, op_add, op_max
                    )

            # ---- L3: 128x32 col-tiled, 4 bands in one psum bank --------
            def l3_part(g, js):
                psy = psy_of_group[g]
                for tt in range(4):
                    for j in js:
                        a2c = a2_of_chunk[2 * g + j // 2]
                        # sim's psum group check is zero-region-coarse; the
                        # four col bands accumulate independently on HW
                        # (per-element has_written), so skip it
                        nc.tensor.matmul(
                            psy[32 * j : 32 * j + 32, :],
                            w3[:, tt, :],
                            a2c[:, j % 2, tt, :],
                            start=(tt == 0), stop=(tt == 3),
                            tile_position=(0, 32 * j),
                            skip_group_check=True,
                        )

            def tanh_half(g, half):
                psy = psy_of_group[g]
                pb = 64 * half
                yt = outs.tile([64, TILE], F32, tag=f"yt{half}", name=f"yt{half}")
                nc.scalar.activation(
                    yt[:], psy[pb : pb + 64, :], tanh,
                    bias=biasc[pb : pb + 64, 8:9],
                )
                if yt_of_group[g] is None:
                    yt_of_group[g] = [None, None]
                yt_of_group[g][half] = yt

            def emit_half(g, half, queues=None):
                queues = queues or (nc.sync, nc.sync)
                ytT = outs.tile([64, TILE], F32, tag=f"ytT{half}", name=f"ytT{half}")
                nc.vector.transpose(ytT[:], yt_of_group[g][half][:])
                for jj in range(2):
                    row0 = (4 * g + 2 * half + jj) * TILE
                    queues[jj].dma_start(
                        out_rows[row0 : row0 + TILE].rearrange(
                            "(k c) m -> c k m", c=32
                        ),
                        ytT[32 * jj : 32 * jj + 32, :].rearrange(
                            "p (k i) -> p k i", k=16
                        )[:, :, 0:M],
                    )

            # ---- steady-state chunk streams ----------------------------
            # two slots per chunk, each [pair, pair, G, G, G, G]: pairs are
            # CLUSTERED two-at-a-time because a full-128 LDWEIGHTS cannot
            # pull ahead past an in-flight row-tiled matmul -- every
            # pair<->L2 boundary pays ~95ns, so fewer boundaries win.
            # build_xt(c+1) is emitted mid-stream so the DVE FIFO never
            # parks it behind late-psum evacs; tanh halves are split across
            # adjacent chunks (A in even chunks' slot1, B at the next odd
            # chunk's slot0 head, always before that chunk's l3 reuses the
            # psy bank).
            L2_ENG = {0: "S", 1: "V", 2: "S", 3: "S",
                      4: "S", 5: "V", 6: "S", 7: "S"}
            build_xt(0)
            for c in range(NCH):
                a1_of_chunk[c] = a1p.tile(
                    [128, 4, 2, TILE], F16, tag="a1", name="a1c"
                )
                a2_of_chunk[c] = a2p.tile(
                    [128, 2, 4, TILE], F16, tag="a2", name="a2c"
                )
                # slot 0: pairs s0,s1 + G0..G3 of c-1
                if c >= 5 and c % 2 == 1:
                    g = (c - 5) // 2
                    tanh_half(g, 1)
                    emit_half(g, 1)
                psp0 = pair(c, 0)
                psp1 = pair(c, 1)
                # chunk 0's evacs all ride scalar so the DVE can run the
                # first three builds back-to-back during the ramp
                evac_pair(c, 0, psp0, eng=("S" if c == 0 else "V"))
                evac_pair(c, 1, psp1, eng="S")
                if c >= 1:
                    for k in range(4):
                        l2_group(c - 1, k, L2_ENG[k])
                if c + 1 < NCH:
                    build_xt(c + 1)
                # slot 1: pairs s2,s3 + G4..G7 of c-1 (+ l3 on odd chunks)
                if c >= 4 and c % 2 == 0:
                    g = (c - 4) // 2
                    tanh_half(g, 0)
                    emit_half(g, 0)
                psp2 = pair(c, 2)
                psp3 = pair(c, 3)
                evac_pair(c, 2, psp2, eng=("S" if c == 0 else "V"))
                evac_pair(c, 3, psp3, eng="S")
                if c >= 1:
                    for k in range(4, 8):
                        l2_group(c - 1, k, L2_ENG[k])
                if c >= 3 and c % 2 == 1:
                    g = (c - 3) // 2
                    psy_of_group[g] = pyp.tile(
                        [128, TILE], F32, tag="psy", name="psy"
                    )
                    l3_part(g, (0, 1, 2, 3))
                if c == 0:
                    warm(N_FILL)
                if c + 4 < NCH:
                    build_dma(c + 4)

            # ---- drain: l2(15), l3(7) split, tanh(6,7) -----------------
            tanh_half(6, 0)
            tanh_half(6, 1)
            emit_half(6, 0)
            emit_half(6, 1)
            psy_of_group[7] = pyp.tile([128, TILE], F32, tag="psy", name="psyF")
            l2_group(15, 0, "S")
            l2_group(15, 1, "V")
            # bands 0,1 need only chunk 14's a2 -- finish + store half A
            # while the rest of chunk 15's L2 still runs
            l3_part(7, (0, 1))
            tanh_half(7, 0)
            emit_half(7, 0)
            l2_group(15, 2, "S")
            l2_group(15, 3, "V")
            l3_part(7, (2,))
            l2_group(15, 4, "S")
            l2_group(15, 5, "V")
            l2_group(15, 6, "S")
            l2_group(15, 7, "V")
            l3_part(7, (3,))
            tanh_half(7, 1)
            emit_half(7, 1, queues=(nc.sync, nc.scalar))

    nc.compile()
    return nc


def _pack_weights(np_in):
    W1 = np.asarray(np_in["W1"], np.float32)
    W2 = np.asarray(np_in["W2"], np.float32)
    W3 = np.asarray(np_in["W3"], np.float32)
    b1 = np.asarray(np_in["b1"], np.float32)
    b2 = np.asarray(np_in["b2"], np.float32)
    b3 = np.asarray(np_in["b3"], np.float32)

    # X^T strip rows: [0:16)=h [16:32)=0 [32:48)=c [48:64)=F ; W1 rows are
    # ordered (F 0:16, c 16:32, h 32:48) in the reference
    w1p = np.zeros((128, H1), np.float16)
    w1p[0:16] = W1[32:48]
    w1p[32:48] = W1[16:32]
    w1p[48:64] = W1[0:16]
    w1p[64:128] = w1p[0:64]

    w2p = np.ascontiguousarray(
        W2.reshape(4, 128, H2).transpose(1, 0, 2).astype(np.float16)
    )
    # pad W3 to 32 out cols so each L3 col band writes its full 32 psum
    # partitions (bands 16:32 etc. would otherwise be uninitialized reads
    # for the whole-half tanh; as zeros they tanh to 0 and are dropped)
    w3p = np.zeros((128, 4, 32), np.float16)
    w3p[:, :, 0:M] = W3.reshape(4, 128, M).transpose(1, 0, 2)
    biasp = np.zeros((128, 9), np.float32)
    biasp[:, 0:4] = b1.reshape(4, 128).T
    biasp[:, 4:8] = b2.reshape(4, 128).T
    for j in range(4):
        biasp[32 * j : 32 * j + M, 8] = b3
    return {"w1p": w1p, "w2p": w2p, "w3p": w3p, "biasp": biasp}


def _core_inputs(np_in, i, packed=None):
    if packed is None:
        packed = _pack_weights(np_in)
    sl = slice(i * B_SH, (i + 1) * B_SH)
    return {
        "C": np_in["C"][sl],
        "F": np_in["F"][sl],
        "H": np_in["H"][sl],
        **packed,
    }


def _get_nc():
    if "nc" not in _CACHE:
        _CACHE["nc"] = _build()
    return _CACHE["nc"]


def run(inputs, trace=False):
    nc = _get_nc()
    np_in = {k: np.ascontiguousarray(np.asarray(v, dtype=np.float32))
             for k, v in inputs.items()}
    packed = _pack_weights(np_in)
    in_maps = [_core_inputs(np_in, i, packed) for i in range(N_CORES)]
    res = run_bass_kernel_spmd(nc, in_maps, list(range(N_CORES)), trace=trace)
    out = np.concatenate([res.results[i]["out"] for i in range(N_CORES)], axis=0)
    return out, res


def kernel(**inputs):
    out, _ = run(inputs, trace=False)
    return out


# revision 11
# speedup vs baseline: 1.2124x; 1.2124x over previous
"""Trainium2 Bass kernel for the Antenna message-generation MLP.

Reference computation (per batch b, RF-chain r, antenna u):
    x[b,r,u,:48] = concat(F[b,:,r], sum_u C[b,u,r,:], H[b,u,8r:8r+8], H[b,u,64+8r:64+8r+8])
    out[b,r,u,:] = tanh(relu(relu(x@W1+b1)@W2+b2)@W3+b3)

Strategy: pure data parallelism over batch across 8 NeuronCores (256
batches = 16384 rows per core).  Rows are processed in 1024-row chunks
(two 512-row subtiles A/B), activations feature-on-partition, fp16 on
the PE (fp32 psum).

Differences from the previous 198us version:
  * Weights are packed on the HOST into fp16 device layouts (w1p/w2p/
    w3p + one [128,9] bias pack) -- no SWDGE cast DMAs, no on-chip w1
    shuffling, and b1/b2/b3 ride the ACT bias port so the folded-bias
    ones rows disappear (L1 contraction 48 in a 64-row band).
  * L1 is 2-way ROW-TILED: subtile A's X^T at partitions 0:64 with the
    stationary at array rows 0:64, subtile B at 64:128/(64,0).  The two
    64-contraction matmuls run concurrently on disjoint row bands ->
    half the PE slots of the old zero-padded 128x128 scheme.
  * Emission interleaves each L1 pair with two L2 groups of the
    previous chunk so psum-bank WAR never blocks the PE FIFO head.
  * PSUM: L1 2x two-bank pair tiles, L2 3 banks (the old 2-bank L2
    rotation cost +54ns at every 4-MM group boundary), L3 packs its 4
    column bands (partitions 32j) into ONE bank.
  * Evacuations balanced across scalar/ACT and vector/DVE (Pool can't
    read PSUM): scalar 7 L2 evacs + 2 pair evacs + a tanh half per
    chunk, DVE 2 pair evacs + 1 L2 evac + builds/transposes.
  * Ramp: chunk 0-3 input DMAs spread across sync/vector/scalar/gpsimd
    queues; tail: final group's tanh/store of bands 0:64 overlaps the
    last chunk's L2 groups.

X^T strip layout (per 64-partition half):
    [0:16)=h  [16:32)=zeros  [32:48)=c  [48:64)=F
C/H land via one merged [128,512] DMA + one DVE 32x32 stream transpose;
c is u-summed by a single tensor_reduce and rejoined with DMA-transposed
F in a 32-row fc tile so one broadcast copy fills c+F per strip.
"""

import sys
import types

import numpy as np

# This image's `antenv` lacks `axon_hooks`; bass_utils imports it when
# BASS_TRACE is set.  Register a no-op stand-in so tracing degrades
# gracefully instead of crashing (real hook installed by test harness).
try:
    import antenv.axon_hooks  # noqa: F401
except ImportError:
    import antenv

    _m = types.ModuleType("antenv.axon_hooks")
    _m._hook = None
    _m.set_axon_ntff_profile_hook = lambda h: setattr(_m, "_hook", h)
    _m.get_axon_ntff_profile_hook = lambda: _m._hook
    sys.modules["antenv.axon_hooks"] = _m
    antenv.axon_hooks = _m

import concourse.bacc as bacc
import concourse.mybir as mybir
import concourse.tile as tile
from concourse.bass_utils import run_bass_kernel_spmd

F32 = mybir.dt.float32
F16 = mybir.dt.float16

N_CORES = 8
B_FULL = 2048
B_SH = B_FULL // N_CORES    # 256 batches per core
U = 8
R = 8
M = 16
FDIM = 16
H1 = 512
H2 = 512

BG = 16                     # batches per build chunk (1024 rows)
NCH = B_SH // BG            # 16 chunks per core
TILE = 512                  # rows per subtile / psum bank of fp32

N_WARM = 116                # PE warm-up matmuls before first L1 pair
N_FILL = 96                 # pipeline-fill matmuls after chunk 0's pairs

_CACHE = {}


def _build():
    nc = bacc.Bacc("TRN2", target_bir_lowering=False, debug=False)

    C_ext = nc.dram_tensor("C", [B_SH, U, R, M], F32, kind="ExternalInput")
    F_ext = nc.dram_tensor("F", [B_SH, FDIM, R], F32, kind="ExternalInput")
    H_ext = nc.dram_tensor("H", [B_SH, U, 2 * 64], F32, kind="ExternalInput")
    # host-packed weights (see _pack_weights)
    w1_ext = nc.dram_tensor("w1p", [128, H1], F16, kind="ExternalInput")
    w2_ext = nc.dram_tensor("w2p", [128, 4, H2], F16, kind="ExternalInput")
    w3_ext = nc.dram_tensor("w3p", [128, 4, 32], F16, kind="ExternalInput")
    # cols 0:4 = b1 (by s-tile), 4:8 = b2 (by t-tile), 8 = b3 (banded)
    bias_ext = nc.dram_tensor("biasp", [128, 9], F32, kind="ExternalInput")
    out_ext = nc.dram_tensor("out", [B_SH, R, U, M], F32, kind="ExternalOutput")

    out_rows = out_ext.ap().rearrange("b r u m -> (b r u) m")  # [16384, 16]

    relu = mybir.ActivationFunctionType.Relu
    tanh = mybir.ActivationFunctionType.Tanh
    axis_x = mybir.AxisListType.X
    op_add = mybir.AluOpType.add
    op_max = mybir.AluOpType.max

    with tile.TileContext(nc) as tc:
        with (
            tc.tile_pool(name="consts", bufs=1) as consts,
            tc.tile_pool(name="loads", bufs=6) as loads,
            tc.tile_pool(name="mts", bufs=3) as mts,
            tc.tile_pool(name="fcs", bufs=6) as fcs,
            tc.tile_pool(name="a1s", bufs=3) as a1p,
            tc.tile_pool(name="a2s", bufs=4) as a2p,
            tc.tile_pool(name="outs", bufs=2) as outs,
            tc.tile_pool(name="p1", bufs=2, space="PSUM") as p1p,
            tc.tile_pool(name="p2", bufs=3, space="PSUM") as p2p,
            tc.tile_pool(name="py", bufs=1, space="PSUM") as pyp,
        ):
            # ---- persistent tiles --------------------------------------
            w1 = consts.tile([128, H1], F16)
            w2 = consts.tile([128, 4, H2], F16)
            w3 = consts.tile([128, 4, 32], F16)
            biasc = consts.tile([128, 9], F32)
            wtile = consts.tile([128, 128], F16)
            wscr = consts.tile([128, 1], F32)
            xts = [consts.tile([128, TILE], F16, tag=f"xt{i}", name=f"xt{i}")
                   for i in range(4)]

            mpads = []
            fc_tiles = []

            def build_dma(c, qc=None, qh=None, qf=None):
                qc = qc or nc.sync
                qh = qh or nc.sync
                qf = qf or nc.sync
                b0 = c * BG
                mp = loads.tile([128, 512], F32, tag="mpad", name="mpad")
                # the DMAs below only fill the lower 16 cols of each 32-col
                # r-block; zero the upper halves so the full-tile DVE
                # transpose never reads uninitialized SBUF (the transposed
                # garbage bands are discarded, but CoreSim checks reads)
                mpv = mp.rearrange("p (r w m) -> p r w m", r=2 * R, w=2)
                nc.gpsimd.memset(mpv[:, :, 1, :], 0.0)
                # c-region: cols 32r + m (m<16)
                qc.dma_start(
                    mp[:, 0:256].rearrange("p (r w) -> p r w", r=R)[:, :, 0:M],
                    C_ext[b0 : b0 + BG].rearrange("b u r m -> (b u) r m"),
                )
                # h-region: cols 256 + 32r + 8i + k
                hp_v = mp[:, 256:512].rearrange("p (r w) -> p r w", r=R)
                h_src = H_ext[b0 : b0 + BG].rearrange(
                    "b u (i r k) -> (b u) i r k", i=2, r=R
                )
                for i in range(2):
                    qh.dma_start(hp_v[:, :, 8 * i : 8 * i + 8], h_src[:, i])
                # F slice straight into fc rows 16:32 (DMA writes any base)
                fcv = fcs.tile([32, 128], F32, tag="fc", name="fc")
                qf.dma_start(
                    fcv[16:32, :].rearrange("f (b r) -> f b r", b=BG),
                    F_ext[b0 : b0 + BG].rearrange("b f r -> f b r"),
                )
                mpads.append(mp)
                fc_tiles.append(fcv)

            # ---- ramp: all input + weight DMAs first, spread over the
            # three DMA-capable queues (sync/SP, scalar/ACT, gpsimd) ----
            # xt pad rows must be finite zeros once; memsets first on the
            # gpsimd queue so build(0)'s xt writes aren't stuck behind its
            # SWDGE descriptor generation
            nc.gpsimd.memset(wtile[:], 0.0)
            for xt in xts:
                nc.gpsimd.memset(xt[:], 0.0)
            build_dma(0, qc=nc.sync, qh=nc.scalar, qf=nc.scalar)
            nc.sync.dma_start(w1[:], w1_ext.ap())
            build_dma(1, qc=nc.scalar, qh=nc.sync, qf=nc.sync)
            # hoist the ~2.7us ACT table load off the critical path
            nc.scalar.activation(wscr[:], wtile[:, 0:1], tanh)
            nc.scalar.dma_start(w2[:], w2_ext.ap())
            nc.sync.dma_start(w3[:], w3_ext.ap())
            nc.sync.dma_start(biasc[:], bias_ext.ap())
            build_dma(2, qc=nc.sync, qh=nc.sync, qf=nc.scalar)
            build_dma(3, qc=nc.sync, qh=nc.scalar, qf=nc.scalar)

            # ---- PE warm-up: keep HAM busy through the input ramp ------
            ps_warm = pyp.tile([128, TILE], F32, tag="psy", name="ps_warm")

            def warm(n):
                for _ in range(n):
                    nc.tensor.matmul(
                        ps_warm[:, 0:64], wtile[:], wtile[:, 0:64],
                        start=True, stop=True,
                    )

            warm(N_WARM)

            # ---- per-chunk build ---------------------------------------
            a1_of_chunk = [None] * NCH
            a2_of_chunk = [None] * NCH
            psy_of_group = [None] * (NCH // 2)
            yt_of_group = [None] * (NCH // 2)

            def build_xt(c, cast_eng="V"):
                mp = mpads[c]
                fcv = fc_tiles[c]
                xt = xts[c % 4]
                mt = mts.tile([128, 512], F32, tag="mt", name="mt")
                nc.vector.transpose(mt[:], mp[:])
                # u-sum of c across all four 32-row bands at once
                cr = mts.tile([128, 32], F32, tag="cred", name="cred")
                nc.vector.tensor_reduce(
                    cr[:],
                    mt[:, 0:256].rearrange("p (rb u) -> p rb u", u=U),
                    axis_x, op_add,
                )
                # c bands -> fc rows 0:16 (cols (b,r) b-major)
                for a in range(4):
                    nc.vector.tensor_copy(
                        fcv[0:16, 32 * a : 32 * a + 32].rearrange(
                            "p (b4 r) -> p r b4", b4=4
                        ),
                        cr[32 * a : 32 * a + 16, :].rearrange(
                            "p (r b4) -> p r b4", b4=4
                        ),
                    )
                # h bands -> xt[0:16) / xt[64:80).  cast_eng="S" rides the
                # scalar ACT (used for chunks 1-2 only: the scalar queue is
                # idle during the ramp while the DVE owes two whole builds
                # before steady state -- gpsimd is NOT an option, its HW
                # cast rate is ~1.3us/op)
                def cast_copy(dst, srcv):
                    if cast_eng == "S":
                        nc.scalar.copy(dst, srcv)
                    else:
                        nc.vector.tensor_copy(dst, srcv)

                for a in range(4):
                    hb = 0 if a < 2 else 64
                    dst = xt[hb : hb + 16, :].rearrange(
                        "p (b r u) -> p b r u", b=8, r=R
                    )[:, 4 * (a & 1) : 4 * (a & 1) + 4]
                    src = mt[32 * a : 32 * a + 16, 256:512].rearrange(
                        "p (r b4 u) -> p b4 r u", b4=4, u=U
                    )
                    cast_copy(dst, src)
                # fc ([c;F], 32 rows) broadcast over u -> xt[32:64)/[96:128)
                for half in range(2):
                    cast_copy(
                        xt[32 + 64 * half : 64 + 64 * half, :].rearrange(
                            "p (b r u) -> p b r u", b=8, r=R
                        ),
                        fcv[:, 64 * half : 64 * half + 64]
                        .rearrange("p (b r) -> p b r", b=8)
                        .unsqueeze(3)
                        .broadcast_to((32, 8, R, U)),
                    )

            # ---- L1: 2-way row-tiled pair ------------------------------
            def pair(c, s):
                xt = xts[c % 4]
                psp = p1p.tile([128, 2, TILE], F32, tag="ps1", name="psp")
                for half in range(2):
                    pb = 64 * half
                    nc.tensor.matmul(
                        psp[:, half, :],
                        w1[pb : pb + 64, s * 128 : (s + 1) * 128],
                        xt[pb : pb + 64, :],
                        start=True, stop=True,
                    )
                return psp

            def evac_pair(c, s, psp, eng="S"):
                a1c = a1_of_chunk[c]
                if eng == "S":
                    nc.scalar.activation(
                        a1c[:, s, :, :], psp[:, :, :], relu,
                        bias=biasc[:, s : s + 1],
                    )
                else:
                    nc.vector.tensor_scalar(
                        a1c[:, s, :, :], psp[:, :, :],
                        biasc[:, s : s + 1], 0.0, op_add, op_max,
                    )

            # ---- L2: group k = (h, t), 4-MM accumulation ---------------
            # (gpsimd/Pool cannot touch PSUM on trn2, so evacuations are
            # spread over scalar/ACT and vector/DVE only)
            def l2_group(c, k, eng="S"):
                h, t = k // 4, k % 4
                a1c = a1_of_chunk[c]
                ps2 = p2p.tile([128, TILE], F32, tag="ps2", name="ps2")
                for s in range(4):
                    nc.tensor.matmul(
                        ps2[:],
                        w2[:, s, t * 128 : (t + 1) * 128],
                        a1c[:, s, h, :],
                        start=(s == 0), stop=(s == 3),
                    )
                a2c = a2_of_chunk[c]
                dst = a2c[:, h, t, :]
                bcol = biasc[:, 4 + t : 5 + t]
                if eng == "S":
                    nc.scalar.activation(dst, ps2[:], relu, bias=bcol)
                else:
                    nc.vector.tensor_scalar(
                        dst, ps2[:], bcol, 0.0, op_add, op_max
                    )

            # ---- L3: 128x32 col-tiled, 4 bands in one psum bank --------
            def l3_part(g, js):
                psy = psy_of_group[g]
                for tt in range(4):
                    for j in js:
                        a2c = a2_of_chunk[2 * g + j // 2]
                        # sim's psum group check is zero-region-coarse; the
                        # four col bands accumulate independently on HW
                        # (per-element has_written), so skip it
                        nc.tensor.matmul(
                            psy[32 * j : 32 * j + 32, :],
                            w3[:, tt, :],
                            a2c[:, j % 2, tt, :],
                            start=(tt == 0), stop=(tt == 3),
                            tile_position=(0, 32 * j),
                            skip_group_check=True,
                        )

            def tanh_full(g):
                psy = psy_of_group[g]
                yt = outs.tile([128, TILE], F32, tag="ytF", name="ytF")
                nc.scalar.activation(
                    yt[:], psy[:, :], tanh, bias=biasc[:, 8:9]
                )
                yt_of_group[g] = yt

            def emit_full(g):
                ytT = outs.tile([128, TILE], F32, tag="ytTF", name="ytTF")
                nc.vector.transpose(ytT[:], yt_of_group[g][:])
                for jj in range(4):
                    row0 = (4 * g + jj) * TILE
                    nc.sync.dma_start(
                        out_rows[row0 : row0 + TILE].rearrange(
                            "(k c) m -> c k m", c=32
                        ),
                        ytT[32 * jj : 32 * jj + 32, :].rearrange(
                            "p (k i) -> p k i", k=16
                        )[:, :, 0:M],
                    )

            def tanh_half(g, half):
                psy = psy_of_group[g]
                pb = 64 * half
                yt = outs.tile([64, TILE], F32, tag=f"yt{half}", name=f"yt{half}")
                nc.scalar.activation(
                    yt[:], psy[pb : pb + 64, :], tanh,
                    bias=biasc[pb : pb + 64, 8:9],
                )
                if not isinstance(yt_of_group[g], list):
                    yt_of_group[g] = [None, None]
                yt_of_group[g][half] = yt

            def emit_half(g, half, queues=None):
                queues = queues or (nc.sync, nc.sync)
                ytT = outs.tile([64, TILE], F32, tag=f"ytT{half}", name=f"ytT{half}")
                nc.vector.transpose(ytT[:], yt_of_group[g][half][:])
                for jj in range(2):
                    row0 = (4 * g + 2 * half + jj) * TILE
                    queues[jj].dma_start(
                        out_rows[row0 : row0 + TILE].rearrange(
                            "(k c) m -> c k m", c=32
                        ),
                        ytT[32 * jj : 32 * jj + 32, :].rearrange(
                            "p (k i) -> p k i", k=16
                        )[:, :, 0:M],
                    )

            # ---- steady-state chunk streams ----------------------------
            # two slots per chunk, each [pair, pair, G, G, G, G]: pairs are
            # CLUSTERED two-at-a-time because a full-128 LDWEIGHTS cannot
            # pull ahead past an in-flight row-tiled matmul -- every
            # pair<->L2 boundary pays ~95ns, so fewer boundaries win.
            # build_xt(c+1) is emitted mid-stream so the DVE FIFO never
            # parks it behind late-psum evacs; tanh halves are split across
            # adjacent chunks (A in even chunks' slot1, B at the next odd
            # chunk's slot0 head, always before that chunk's l3 reuses the
            # psy bank).
            L2_ENG = {0: "S", 1: "S", 2: "S", 3: "S",
                      4: "S", 5: "V", 6: "S", 7: "S"}
            build_xt(0)
            for c in range(NCH):
                a1_of_chunk[c] = a1p.tile(
                    [128, 4, 2, TILE], F16, tag="a1", name="a1c"
                )
                a2_of_chunk[c] = a2p.tile(
                    [128, 2, 4, TILE], F16, tag="a2", name="a2c"
                )
                # slot 0: pairs s0,s1 + G0..G3 of c-1
                psp0 = pair(c, 0)
                psp1 = pair(c, 1)
                evac_pair(c, 0, psp0, eng=("S" if c == 0 else "V"))
                evac_pair(c, 1, psp1, eng="S")
                if c >= 1:
                    for k in range(4):
                        l2_group(c - 1, k, L2_ENG[k])
                if c + 1 < NCH:
                    # chunks 1-2's casts ride the idle scalar queue so the
                    # DVE enters steady state without ramp debt
                    build_xt(c + 1, cast_eng=("S" if c <= 1 else "V"))
                # slot 1: pairs s2,s3 + G4..G7 of c-1 (+ l3 on odd chunks)
                if c >= 4 and c % 2 == 0:
                    g = (c - 4) // 2
                    tanh_full(g)
                    emit_full(g)
                psp2 = pair(c, 2)
                psp3 = pair(c, 3)
                evac_pair(c, 2, psp2, eng="V")
                evac_pair(c, 3, psp3, eng="S")
                if c >= 1:
                    for k in range(4, 8):
                        l2_group(c - 1, k, L2_ENG[k])
                if c >= 3 and c % 2 == 1:
                    g = (c - 3) // 2
                    psy_of_group[g] = pyp.tile(
                        [128, TILE], F32, tag="psy", name="psy"
                    )
                    l3_part(g, (0, 1, 2, 3))
                if c == 0:
                    warm(N_FILL)
                if c + 4 < NCH:
                    build_dma(c + 4)

            # ---- drain: l2(15), l3(7) split, tanh(6,7) -----------------
            tanh_full(6)
            emit_full(6)
            psy_of_group[7] = pyp.tile([128, TILE], F32, tag="psy", name="psyF")
            l2_group(15, 0, "S")
            l2_group(15, 1, "V")
            # bands 0,1 need only chunk 14's a2 -- finish + store half A
            # while the rest of chunk 15's L2 still runs
            l3_part(7, (0, 1))
            tanh_half(7, 0)
            emit_half(7, 0)
            l2_group(15, 2, "S")
            l2_group(15, 3, "V")
            l3_part(7, (2,))
            l2_group(15, 4, "S")
            l2_group(15, 5, "V")
            l2_group(15, 6, "S")
            l2_group(15, 7, "V")
            l3_part(7, (3,))
            tanh_half(7, 1)
            emit_half(7, 1, queues=(nc.sync, nc.scalar))

    nc.compile()
    return nc


def _pack_weights(np_in):
    W1 = np.asarray(np_in["W1"], np.float32)
    W2 = np.asarray(np_in["W2"], np.float32)
    W3 = np.asarray(np_in["W3"], np.float32)
    b1 = np.asarray(np_in["b1"], np.float32)
    b2 = np.asarray(np_in["b2"], np.float32)
    b3 = np.asarray(np_in["b3"], np.float32)

    # X^T strip rows: [0:16)=h [16:32)=0 [32:48)=c [48:64)=F ; W1 rows are
    # ordered (F 0:16, c 16:32, h 32:48) in the reference
    w1p = np.zeros((128, H1), np.float16)
    w1p[0:16] = W1[32:48]
    w1p[32:48] = W1[16:32]
    w1p[48:64] = W1[0:16]
    w1p[64:128] = w1p[0:64]

    w2p = np.ascontiguousarray(
        W2.reshape(4, 128, H2).transpose(1, 0, 2).astype(np.float16)
    )
    # pad W3 to 32 out cols so each L3 col band writes its full 32 psum
    # partitions (bands 16:32 etc. would otherwise be uninitialized reads
    # for the whole-half tanh; as zeros they tanh to 0 and are dropped)
    w3p = np.zeros((128, 4, 32), np.float16)
    w3p[:, :, 0:M] = W3.reshape(4, 128, M).transpose(1, 0, 2)
    biasp = np.zeros((128, 9), np.float32)
    biasp[:, 0:4] = b1.reshape(4, 128).T
    biasp[:, 4:8] = b2.reshape(4, 128).T
    for j in range(4):
        biasp[32 * j : 32 * j + M, 8] = b3
    return {"w1p": w1p, "w2p": w2p, "w3p": w3p, "biasp": biasp}


def _core_inputs(np_in, i, packed=None):
    if packed is None:
        packed = _pack_weights(np_in)
    sl = slice(i * B_SH, (i + 1) * B_SH)
    return {
        "C": np_in["C"][sl],
        "F": np_in["F"][sl],
        "H": np_in["H"][sl],
        **packed,
    }


def _get_nc():
    if "nc" not in _CACHE:
        _CACHE["nc"] = _build()
    return _CACHE["nc"]


def run(inputs, trace=False):
    nc = _get_nc()
    np_in = {k: np.ascontiguousarray(np.asarray(v, dtype=np.float32))
             for k, v in inputs.items()}
    packed = _pack_weights(np_in)
    in_maps = [_core_inputs(np_in, i, packed) for i in range(N_CORES)]
    res = run_bass_kernel_spmd(nc, in_maps, list(range(N_CORES)), trace=trace)
    out = np.concatenate([res.results[i]["out"] for i in range(N_CORES)], axis=0)
    return out, res


def kernel(**inputs):
    out, _ = run(inputs, trace=False)
    return out


# revision 12
# speedup vs baseline: 1.2144x; 1.0017x over previous
"""Trainium2 Bass kernel for the Antenna message-generation MLP.

Reference computation (per batch b, RF-chain r, antenna u):
    x[b,r,u,:48] = concat(F[b,:,r], sum_u C[b,u,r,:], H[b,u,8r:8r+8], H[b,u,64+8r:64+8r+8])
    out[b,r,u,:] = tanh(relu(relu(x@W1+b1)@W2+b2)@W3+b3)

Strategy: pure data parallelism over batch across 8 NeuronCores (256
batches = 16384 rows per core).  Rows are processed in 1024-row chunks
(two 512-row subtiles A/B), activations feature-on-partition, fp16 on
the PE (fp32 psum).

Differences from the previous 198us version:
  * Weights are packed on the HOST into fp16 device layouts (w1p/w2p/
    w3p + one [128,9] bias pack) -- no SWDGE cast DMAs, no on-chip w1
    shuffling, and b1/b2/b3 ride the ACT bias port so the folded-bias
    ones rows disappear (L1 contraction 48 in a 64-row band).
  * L1 is 2-way ROW-TILED: subtile A's X^T at partitions 0:64 with the
    stationary at array rows 0:64, subtile B at 64:128/(64,0).  The two
    64-contraction matmuls run concurrently on disjoint row bands ->
    half the PE slots of the old zero-padded 128x128 scheme.
  * Emission interleaves each L1 pair with two L2 groups of the
    previous chunk so psum-bank WAR never blocks the PE FIFO head.
  * PSUM: L1 2x two-bank pair tiles, L2 3 banks (the old 2-bank L2
    rotation cost +54ns at every 4-MM group boundary), L3 packs its 4
    column bands (partitions 32j) into ONE bank.
  * Evacuations balanced across scalar/ACT and vector/DVE (Pool can't
    read PSUM): scalar 7 L2 evacs + 2 pair evacs + a tanh half per
    chunk, DVE 2 pair evacs + 1 L2 evac + builds/transposes.
  * Ramp: chunk 0-3 input DMAs spread across sync/vector/scalar/gpsimd
    queues; tail: final group's tanh/store of bands 0:64 overlaps the
    last chunk's L2 groups.

X^T strip layout (per 64-partition half):
    [0:16)=h  [16:32)=zeros  [32:48)=c  [48:64)=F
C/H land via one merged [128,512] DMA + one DVE 32x32 stream transpose;
c is u-summed by a single tensor_reduce and rejoined with DMA-transposed
F in a 32-row fc tile so one broadcast copy fills c+F per strip.
"""

import sys
import types

import numpy as np

# This image's `antenv` lacks `axon_hooks`; bass_utils imports it when
# BASS_TRACE is set.  Register a no-op stand-in so tracing degrades
# gracefully instead of crashing (real hook installed by test harness).
try:
    import antenv.axon_hooks  # noqa: F401
except ImportError:
    import antenv

    _m = types.ModuleType("antenv.axon_hooks")
    _m._hook = None
    _m.set_axon_ntff_profile_hook = lambda h: setattr(_m, "_hook", h)
    _m.get_axon_ntff_profile_hook = lambda: _m._hook
    sys.modules["antenv.axon_hooks"] = _m
    antenv.axon_hooks = _m

import concourse.bacc as bacc
import concourse.mybir as mybir
import concourse.tile as tile
from concourse.bass_utils import run_bass_kernel_spmd

F32 = mybir.dt.float32
F16 = mybir.dt.float16

N_CORES = 8
B_FULL = 2048
B_SH = B_FULL // N_CORES    # 256 batches per core
U = 8
R = 8
M = 16
FDIM = 16
H1 = 512
H2 = 512

BG = 16                     # batches per build chunk (1024 rows)
NCH = B_SH // BG            # 16 chunks per core
TILE = 512                  # rows per subtile / psum bank of fp32

N_WARM = 144                # PE warm-up matmuls before first L1 pair
N_FILL = 104                # pipeline-fill matmuls after chunk 0's pairs

# CoreSim flags uninitialized reads; the mpad upper-half zeroing exists only
# for that check (the transposed garbage bands are never consumed).  HW runs
# skip it -- test.py verifies numerics on hardware either way.
SIM_SAFE = False

_CACHE = {}


def _build():
    nc = bacc.Bacc("TRN2", target_bir_lowering=False, debug=False)

    C_ext = nc.dram_tensor("C", [B_SH, U, R, M], F32, kind="ExternalInput")
    F_ext = nc.dram_tensor("F", [B_SH, FDIM, R], F32, kind="ExternalInput")
    H_ext = nc.dram_tensor("H", [B_SH, U, 2 * 64], F32, kind="ExternalInput")
    # host-packed weights (see _pack_weights)
    w1_ext = nc.dram_tensor("w1p", [128, H1], F16, kind="ExternalInput")
    w2_ext = nc.dram_tensor("w2p", [128, 4, H2], F16, kind="ExternalInput")
    w3_ext = nc.dram_tensor("w3p", [128, 4, 32], F16, kind="ExternalInput")
    # cols 0:4 = b1 (by s-tile), 4:8 = b2 (by t-tile), 8 = b3 (banded)
    bias_ext = nc.dram_tensor("biasp", [128, 9], F32, kind="ExternalInput")
    out_ext = nc.dram_tensor("out", [B_SH, R, U, M], F32, kind="ExternalOutput")

    out_rows = out_ext.ap().rearrange("b r u m -> (b r u) m")  # [16384, 16]

    relu = mybir.ActivationFunctionType.Relu
    tanh = mybir.ActivationFunctionType.Tanh
    axis_x = mybir.AxisListType.X
    op_add = mybir.AluOpType.add
    op_max = mybir.AluOpType.max

    with tile.TileContext(nc) as tc:
        with (
            tc.tile_pool(name="consts", bufs=1) as consts,
            tc.tile_pool(name="loads", bufs=6) as loads,
            tc.tile_pool(name="mts", bufs=3) as mts,
            tc.tile_pool(name="fcs", bufs=6) as fcs,
            tc.tile_pool(name="a1s", bufs=3) as a1p,
            tc.tile_pool(name="a2s", bufs=4) as a2p,
            tc.tile_pool(name="outs", bufs=2) as outs,
            tc.tile_pool(name="p1", bufs=2, space="PSUM") as p1p,
            tc.tile_pool(name="p2", bufs=3, space="PSUM") as p2p,
            tc.tile_pool(name="py", bufs=1, space="PSUM") as pyp,
        ):
            # ---- persistent tiles --------------------------------------
            w1 = consts.tile([128, H1], F16)
            w2 = consts.tile([128, 4, H2], F16)
            w3 = consts.tile([128, 4, 32], F16)
            biasc = consts.tile([128, 9], F32)
            wtile = consts.tile([128, 128], F16)
            wscr = consts.tile([128, 1], F32)
            xts = [consts.tile([128, TILE], F16, tag=f"xt{i}", name=f"xt{i}")
                   for i in range(4)]

            mpads = []
            fc_tiles = []

            def build_dma(c, qc=None, qh=None, qf=None):
                qc = qc or nc.sync
                qh = qh or nc.sync
                qf = qf or nc.sync
                b0 = c * BG
                mp = loads.tile([128, 512], F32, tag="mpad", name="mpad")
                if SIM_SAFE:
                    # the DMAs below only fill the lower 16 cols of each
                    # 32-col r-block; zero the upper halves so the full-tile
                    # DVE transpose never reads uninitialized SBUF (the
                    # garbage bands are discarded, but CoreSim checks reads)
                    mpv = mp.rearrange("p (r w m) -> p r w m", r=2 * R, w=2)
                    nc.gpsimd.memset(mpv[:, :, 1, :], 0.0)
                # c-region: cols 32r + m (m<16)
                qc.dma_start(
                    mp[:, 0:256].rearrange("p (r w) -> p r w", r=R)[:, :, 0:M],
                    C_ext[b0 : b0 + BG].rearrange("b u r m -> (b u) r m"),
                )
                # h-region: cols 256 + 32r + 8i + k
                hp_v = mp[:, 256:512].rearrange("p (r w) -> p r w", r=R)
                h_src = H_ext[b0 : b0 + BG].rearrange(
                    "b u (i r k) -> (b u) i r k", i=2, r=R
                )
                for i in range(2):
                    qh.dma_start(hp_v[:, :, 8 * i : 8 * i + 8], h_src[:, i])
                # F slice straight into fc rows 16:32 (DMA writes any base)
                fcv = fcs.tile([32, 128], F32, tag="fc", name="fc")
                qf.dma_start(
                    fcv[16:32, :].rearrange("f (b r) -> f b r", b=BG),
                    F_ext[b0 : b0 + BG].rearrange("b f r -> f b r"),
                )
                mpads.append(mp)
                fc_tiles.append(fcv)

            # ---- ramp: all input + weight DMAs first, spread over the
            # three DMA-capable queues (sync/SP, scalar/ACT, gpsimd) ----
            # chunk 0 rides sync alone (shortest path to the first build);
            # chunk 1 rides scalar behind the hoisted ACT table load;
            # chunks 2/3 fill the remaining slots, gpsimd taking two SWDGE
            # transfers since it only has ~1.3us of memsets otherwise
            nc.gpsimd.memset(wtile[:], 0.0)
            for xt in xts:
                nc.gpsimd.memset(xt[:], 0.0)
            # hoist the ~2.7us ACT table load off the critical path
            nc.scalar.activation(wscr[:], wtile[:, 0:1], tanh)
            build_dma(0, qc=nc.sync, qh=nc.sync, qf=nc.sync)
            nc.sync.dma_start(w1[:], w1_ext.ap())
            build_dma(1, qc=nc.scalar, qh=nc.scalar, qf=nc.scalar)
            nc.scalar.dma_start(w2[:], w2_ext.ap())
            nc.sync.dma_start(w3[:], w3_ext.ap())
            nc.sync.dma_start(biasc[:], bias_ext.ap())
            build_dma(2, qc=nc.sync, qh=nc.sync, qf=nc.sync)
            build_dma(3, qc=nc.gpsimd, qh=nc.scalar, qf=nc.gpsimd)

            # ---- PE warm-up: keep HAM busy through the input ramp ------
            ps_warm = pyp.tile([128, TILE], F32, tag="psy", name="ps_warm")

            def warm(n):
                for _ in range(n):
                    nc.tensor.matmul(
                        ps_warm[:, 0:64], wtile[:], wtile[:, 0:64],
                        start=True, stop=True,
                    )

            warm(N_WARM)

            # ---- per-chunk build ---------------------------------------
            a1_of_chunk = [None] * NCH
            a2_of_chunk = [None] * NCH
            psy_of_group = [None] * (NCH // 2)
            yt_of_group = [None] * (NCH // 2)

            def build_xt(c, cast_eng="V"):
                mp = mpads[c]
                fcv = fc_tiles[c]
                xt = xts[c % 4]
                mt = mts.tile([128, 512], F32, tag="mt", name="mt")
                nc.vector.transpose(mt[:], mp[:])
                # u-sum of c across all four 32-row bands at once
                cr = mts.tile([128, 32], F32, tag="cred", name="cred")
                nc.vector.tensor_reduce(
                    cr[:],
                    mt[:, 0:256].rearrange("p (rb u) -> p rb u", u=U),
                    axis_x, op_add,
                )
                # c bands -> fc rows 0:16 (cols (b,r) b-major)
                for a in range(4):
                    nc.vector.tensor_copy(
                        fcv[0:16, 32 * a : 32 * a + 32].rearrange(
                            "p (b4 r) -> p r b4", b4=4
                        ),
                        cr[32 * a : 32 * a + 16, :].rearrange(
                            "p (r b4) -> p r b4", b4=4
                        ),
                    )
                # h bands -> xt[0:16) / xt[64:80).  cast_eng="S" rides the
                # scalar ACT (used for chunks 1-2 only: the scalar queue is
                # idle during the ramp while the DVE owes two whole builds
                # before steady state -- gpsimd is NOT an option, its HW
                # cast rate is ~1.3us/op)
                def cast_copy(dst, srcv):
                    if cast_eng == "S":
                        nc.scalar.copy(dst, srcv)
                    else:
                        nc.vector.tensor_copy(dst, srcv)

                for a in range(4):
                    hb = 0 if a < 2 else 64
                    dst = xt[hb : hb + 16, :].rearrange(
                        "p (b r u) -> p b r u", b=8, r=R
                    )[:, 4 * (a & 1) : 4 * (a & 1) + 4]
                    src = mt[32 * a : 32 * a + 16, 256:512].rearrange(
                        "p (r b4 u) -> p b4 r u", b4=4, u=U
                    )
                    cast_copy(dst, src)
                # fc ([c;F], 32 rows) broadcast over u -> xt[32:64)/[96:128)
                for half in range(2):
                    cast_copy(
                        xt[32 + 64 * half : 64 + 64 * half, :].rearrange(
                            "p (b r u) -> p b r u", b=8, r=R
                        ),
                        fcv[:, 64 * half : 64 * half + 64]
                        .rearrange("p (b r) -> p b r", b=8)
                        .unsqueeze(3)
                        .broadcast_to((32, 8, R, U)),
                    )

            # ---- L1: 2-way row-tiled pair ------------------------------
            def pair(c, s):
                xt = xts[c % 4]
                psp = p1p.tile([128, 2, TILE], F32, tag="ps1", name="psp")
                for half in range(2):
                    pb = 64 * half
                    nc.tensor.matmul(
                        psp[:, half, :],
                        w1[pb : pb + 64, s * 128 : (s + 1) * 128],
                        xt[pb : pb + 64, :],
                        start=True, stop=True,
                    )
                return psp

            def evac_pair(c, s, psp, eng="S"):
                a1c = a1_of_chunk[c]
                if eng == "S":
                    nc.scalar.activation(
                        a1c[:, s, :, :], psp[:, :, :], relu,
                        bias=biasc[:, s : s + 1],
                    )
                else:
                    nc.vector.tensor_scalar(
                        a1c[:, s, :, :], psp[:, :, :],
                        biasc[:, s : s + 1], 0.0, op_add, op_max,
                    )

            # ---- L2: group k = (h, t), 4-MM accumulation ---------------
            # (gpsimd/Pool cannot touch PSUM on trn2, so evacuations are
            # spread over scalar/ACT and vector/DVE only)
            def l2_group(c, k, eng="S"):
                h, t = k // 4, k % 4
                a1c = a1_of_chunk[c]
                ps2 = p2p.tile([128, TILE], F32, tag="ps2", name="ps2")
                for s in range(4):
                    nc.tensor.matmul(
                        ps2[:],
                        w2[:, s, t * 128 : (t + 1) * 128],
                        a1c[:, s, h, :],
                        start=(s == 0), stop=(s == 3),
                    )
                a2c = a2_of_chunk[c]
                dst = a2c[:, h, t, :]
                bcol = biasc[:, 4 + t : 5 + t]
                if eng == "S":
                    nc.scalar.activation(dst, ps2[:], relu, bias=bcol)
                else:
                    nc.vector.tensor_scalar(
                        dst, ps2[:], bcol, 0.0, op_add, op_max
                    )

            # ---- L3: 128x32 col-tiled, 4 bands in one psum bank --------
            def l3_part(g, js):
                psy = psy_of_group[g]
                for tt in range(4):
                    for j in js:
                        a2c = a2_of_chunk[2 * g + j // 2]
                        # sim's psum group check is zero-region-coarse; the
                        # four col bands accumulate independently on HW
                        # (per-element has_written), so skip it
                        nc.tensor.matmul(
                            psy[32 * j : 32 * j + 32, :],
                            w3[:, tt, :],
                            a2c[:, j % 2, tt, :],
                            start=(tt == 0), stop=(tt == 3),
                            tile_position=(0, 32 * j),
                            skip_group_check=True,
                        )

            def tanh_full(g):
                psy = psy_of_group[g]
                yt = outs.tile([128, TILE], F32, tag="ytF", name="ytF")
                nc.scalar.activation(
                    yt[:], psy[:, :], tanh, bias=biasc[:, 8:9]
                )
                yt_of_group[g] = yt

            def emit_full(g):
                ytT = outs.tile([128, TILE], F32, tag="ytTF", name="ytTF")
                nc.vector.transpose(ytT[:], yt_of_group[g][:])
                for jj in range(4):
                    row0 = (4 * g + jj) * TILE
                    nc.sync.dma_start(
                        out_rows[row0 : row0 + TILE].rearrange(
                            "(k c) m -> c k m", c=32
                        ),
                        ytT[32 * jj : 32 * jj + 32, :].rearrange(
                            "p (k i) -> p k i", k=16
                        )[:, :, 0:M],
                    )

            def tanh_half(g, half):
                psy = psy_of_group[g]
                pb = 64 * half
                yt = outs.tile([64, TILE], F32, tag=f"yt{half}", name=f"yt{half}")
                nc.scalar.activation(
                    yt[:], psy[pb : pb + 64, :], tanh,
                    bias=biasc[pb : pb + 64, 8:9],
                )
                if not isinstance(yt_of_group[g], list):
                    yt_of_group[g] = [None, None]
                yt_of_group[g][half] = yt

            def emit_half(g, half, queues=None):
                queues = queues or (nc.sync, nc.sync)
                ytT = outs.tile([64, TILE], F32, tag=f"ytT{half}", name=f"ytT{half}")
                nc.vector.transpose(ytT[:], yt_of_group[g][half][:])
                for jj in range(2):
                    row0 = (4 * g + 2 * half + jj) * TILE
                    queues[jj].dma_start(
                        out_rows[row0 : row0 + TILE].rearrange(
                            "(k c) m -> c k m", c=32
                        ),
                        ytT[32 * jj : 32 * jj + 32, :].rearrange(
                            "p (k i) -> p k i", k=16
                        )[:, :, 0:M],
                    )

            # ---- steady-state chunk streams ----------------------------
            # two slots per chunk, each [pair, pair, G, G, G, G]: pairs are
            # CLUSTERED two-at-a-time because a full-128 LDWEIGHTS cannot
            # pull ahead past an in-flight row-tiled matmul -- every
            # pair<->L2 boundary pays ~95ns, so fewer boundaries win.
            # build_xt(c+1) is emitted mid-stream so the DVE FIFO never
            # parks it behind late-psum evacs; tanh halves are split across
            # adjacent chunks (A in even chunks' slot1, B at the next odd
            # chunk's slot0 head, always before that chunk's l3 reuses the
            # psy bank).
            L2_ENG = {0: "S", 1: "S", 2: "S", 3: "S",
                      4: "S", 5: "V", 6: "S", 7: "S"}
            build_xt(0)
            for c in range(NCH):
                a1_of_chunk[c] = a1p.tile(
                    [128, 4, 2, TILE], F16, tag="a1", name="a1c"
                )
                a2_of_chunk[c] = a2p.tile(
                    [128, 2, 4, TILE], F16, tag="a2", name="a2c"
                )
                # slot 0: pairs s0,s1 + G0..G3 of c-1
                psp0 = pair(c, 0)
                psp1 = pair(c, 1)
                evac_pair(c, 0, psp0, eng=("S" if c == 0 else "V"))
                evac_pair(c, 1, psp1, eng="S")
                if c >= 1:
                    for k in range(4):
                        l2_group(c - 1, k, L2_ENG[k])
                if c + 1 < NCH:
                    # chunks 1-2's casts ride the idle scalar queue so the
                    # DVE enters steady state without ramp debt
                    build_xt(c + 1, cast_eng=("S" if c <= 1 else "V"))
                # slot 1: pairs s2,s3 + G4..G7 of c-1 (+ l3 on odd chunks)
                if c >= 4 and c % 2 == 0:
                    g = (c - 4) // 2
                    tanh_full(g)
                    emit_full(g)
                psp2 = pair(c, 2)
                psp3 = pair(c, 3)
                evac_pair(c, 2, psp2, eng="V")
                evac_pair(c, 3, psp3, eng="S")
                if c >= 1:
                    for k in range(4, 8):
                        l2_group(c - 1, k, L2_ENG[k])
                if c >= 3 and c % 2 == 1:
                    g = (c - 3) // 2
                    psy_of_group[g] = pyp.tile(
                        [128, TILE], F32, tag="psy", name="psy"
                    )
                    l3_part(g, (0, 1, 2, 3))
                if c == 0:
                    warm(N_FILL)
                if c + 4 < NCH:
                    build_dma(c + 4)

            # ---- drain: l2(15), l3(7) split, tanh(6,7) -----------------
            tanh_full(6)
            emit_full(6)
            psy_of_group[7] = pyp.tile([128, TILE], F32, tag="psy", name="psyF")
            l2_group(15, 0, "S")
            l2_group(15, 1, "V")
            # bands 0,1 need only chunk 14's a2 -- finish + store half A
            # while the rest of chunk 15's L2 still runs
            l3_part(7, (0, 1))
            tanh_half(7, 0)
            emit_half(7, 0)
            l2_group(15, 2, "S")
            l2_group(15, 3, "V")
            l3_part(7, (2,))
            l2_group(15, 4, "S")
            l2_group(15, 5, "V")
            l2_group(15, 6, "S")
            l2_group(15, 7, "V")
            l3_part(7, (3,))
            tanh_half(7, 1)
            emit_half(7, 1, queues=(nc.sync, nc.scalar))

    nc.compile()
    return nc


def _pack_weights(np_in):
    W1 = np.asarray(np_in["W1"], np.float32)
    W2 = np.asarray(np_in["W2"], np.float32)
    W3 = np.asarray(np_in["W3"], np.float32)
    b1 = np.asarray(np_in["b1"], np.float32)
    b2 = np.asarray(np_in["b2"], np.float32)
    b3 = np.asarray(np_in["b3"], np.float32)

    # X^T strip rows: [0:16)=h [16:32)=0 [32:48)=c [48:64)=F ; W1 rows are
    # ordered (F 0:16, c 16:32, h 32:48) in the reference
    w1p = np.zeros((128, H1), np.float16)
    w1p[0:16] = W1[32:48]
    w1p[32:48] = W1[16:32]
    w1p[48:64] = W1[0:16]
    w1p[64:128] = w1p[0:64]

    w2p = np.ascontiguousarray(
        W2.reshape(4, 128, H2).transpose(1, 0, 2).astype(np.float16)
    )
    # pad W3 to 32 out cols so each L3 col band writes its full 32 psum
    # partitions (bands 16:32 etc. would otherwise be uninitialized reads
    # for the whole-half tanh; as zeros they tanh to 0 and are dropped)
    w3p = np.zeros((128, 4, 32), np.float16)
    w3p[:, :, 0:M] = W3.reshape(4, 128, M).transpose(1, 0, 2)
    biasp = np.zeros((128, 9), np.float32)
    biasp[:, 0:4] = b1.reshape(4, 128).T
    biasp[:, 4:8] = b2.reshape(4, 128).T
    for j in range(4):
        biasp[32 * j : 32 * j + M, 8] = b3
    return {"w1p": w1p, "w2p": w2p, "w3p": w3p, "biasp": biasp}


def _core_inputs(np_in, i, packed=None):
    if packed is None:
        packed = _pack_weights(np_in)
    sl = slice(i * B_SH, (i + 1) * B_SH)
    return {
        "C": np_in["C"][sl],
        "F": np_in["F"][sl],
        "H": np_in["H"][sl],
        **packed,
    }


def _get_nc():
    if "nc" not in _CACHE:
        _CACHE["nc"] = _build()
    return _CACHE["nc"]


def run(inputs, trace=False):
    nc = _get_nc()
    np_in = {k: np.ascontiguousarray(np.asarray(v, dtype=np.float32))
             for k, v in inputs.items()}
    packed = _pack_weights(np_in)
    in_maps = [_core_inputs(np_in, i, packed) for i in range(N_CORES)]
    res = run_bass_kernel_spmd(nc, in_maps, list(range(N_CORES)), trace=trace)
    out = np.concatenate([res.results[i]["out"] for i in range(N_CORES)], axis=0)
    return out, res


def kernel(**inputs):
    out, _ = run(inputs, trace=False)
    return out
